# revision 18
# baseline (speedup 1.0000x reference)
"""Trainium2 Bass kernel for the dense-MoE problem (nn_MoE_20899310862533).

Contract: kernel(**inputs) takes the FULL unsharded inputs (keys as in
reference.setup_inputs()) and returns the FULL [32768, 256] float32 output.

Strategy: data-parallel over the batch across 8 NeuronCores (weights
replicated, no collectives). On chip everything is kept feature-major
([feature, batch]) so every matmul has its contraction dim on partitions and
a 512-wide moving dim; the host passes x pre-transposed per shard and
transposes the per-core output back.

Per core (batch shard 4096, processed in 8 tiles of 512):
  gating MLP (4 layers, LeakyReLU 0.01) -> softmax -> gates G^T [16, 512]
  experts, all fused in SBUF/PSUM:
      act1_e   = relu(x @ We1[e] + be1[e])          [512h, 512b] per expert
      act1s_e  = act1_e * g_e (broadcast over h)
      psum_out += We2[e].T-contraction with act1s_e  (accumulated over e)
      psum_out += be2.T @ G^T                        (gates-weighted bias)
  Gate rows are broadcast across 128 partitions with a DRAM round-trip
  broadcast DMA. Matmuls run as float32r (full-rate fp32 PE mode).
"""

import numpy as np

import concourse.bass as bass
import concourse.mybir as mybir
import concourse.tile as tile
from concourse import bacc
from concourse import bass_isa
from concourse.bass_utils import run_bass_kernel_spmd

F32 = mybir.dt.float32
F32R = mybir.dt.float32r
BF16 = mybir.dt.bfloat16
AF = mybir.ActivationFunctionType
ALU = mybir.AluOpType
AX = mybir.AxisListType

B, D, H, O, E = 32768, 256, 512, 256, 16
NCORES = 8
BC = B // NCORES      # 4096 batch rows per core
BT = 512              # batch tile (moving/free dim)
NBT = BC // BT        # 8
KD = D // 128         # 2 contraction tiles for D
HT = H // 128         # 4 tiles for H
OT = O // 128         # 2 tiles for O

MMDT = F32R           # matmul operand dtype view (F32R = fast, F32 = exact)


def build_program(n_reps=1, has_be1=False, has_be2=False, has_bg4=False,
                  psum_cfg=(4, 2), unroll=False):
    nc = bacc.Bacc("TRN2", target_bir_lowering=False, debug=False,
                   num_devices=NCORES)

    def din(name, shape, dt=F32):
        return nc.dram_tensor(name, shape, dt, kind="ExternalInput").ap()

    xT = din("xT", [D, BC], MMDT)
    We1 = din("We1", [E, D, H], MMDT)
    We2 = din("We2", [E, H, O], MMDT)
    be1T = din("be1T", [H, E])
    be2 = din("be2", [E, O], MMDT)
    Wg1 = din("Wg1", [D, 128], MMDT)
    Wg2 = din("Wg2", [128, 256], MMDT)
    Wg3 = din("Wg3", [256, 128], MMDT)
    Wg4 = din("Wg4", [128, E], MMDT)
    bg1 = din("bg1", [128, 1])
    bg2 = din("bg2", [256, 1])
    bg3 = din("bg3", [128, 1])
    bg4c = din("bg4c", [E, 1])
    outT = nc.dram_tensor("outT", [O, BC], F32, kind="ExternalOutput").ap()
    gscr = nc.dram_tensor("gscr", [NBT, E, BT], BF16, kind="Internal").ap()

    with tile.TileContext(nc) as tc:
        from contextlib import ExitStack
        with ExitStack() as ctx:
            const = ctx.enter_context(tc.tile_pool(name="const", bufs=1))
            xpool = ctx.enter_context(tc.tile_pool(name="x", bufs=2))
            gpool = ctx.enter_context(tc.tile_pool(name="gate", bufs=1))
            smx = ctx.enter_context(tc.tile_pool(name="smx", bufs=1))
            gtpool = ctx.enter_context(tc.tile_pool(name="gt", bufs=2))
            gbpool = ctx.enter_context(tc.tile_pool(name="gb", bufs=3))
            rpool = ctx.enter_context(tc.tile_pool(name="r1", bufs=3))
            apool = ctx.enter_context(tc.tile_pool(name="a1s", bufs=2))
            opool = ctx.enter_context(tc.tile_pool(name="osb", bufs=2))
            pbig = ctx.enter_context(
                tc.tile_pool(name="pbig", bufs=psum_cfg[0], space="PSUM"))
            pout = ctx.enter_context(
                tc.tile_pool(name="pout", bufs=psum_cfg[1], space="PSUM"))

            def load_x(bt, name="xs", split=1):
                xs = xpool.tile([128, KD, BT], MMDT, name=name)
                w = BT // split
                for kd in range(KD):
                    for s in range(split):
                        nc.sync.dma_start(
                            out=xs[:, kd, s * w:(s + 1) * w],
                            in_=xT[kd * 128:(kd + 1) * 128,
                                   bt * BT + s * w:bt * BT + (s + 1) * w])
                return xs

            # ---- gating-critical constants + first x tile first, so the
            #      PE starts as early as possible ----
            wg1_s = const.tile([128, KD, 128], MMDT, name="wg1_s")
            for kd in range(KD):
                nc.sync.dma_start(out=wg1_s[:, kd, :],
                                  in_=Wg1[kd * 128:(kd + 1) * 128, :])
            bg1_s = const.tile([128, 1], F32, name="bg1_s")
            nc.sync.dma_start(out=bg1_s, in_=bg1)
            xs0 = load_x(0, name="xs0", split=2)
            wg2_s = const.tile([128, 256], MMDT, name="wg2_s")
            nc.sync.dma_start(out=wg2_s, in_=Wg2)
            wg3_s = const.tile([128, KD, 128], MMDT, name="wg3_s")
            for kd in range(KD):
                nc.sync.dma_start(out=wg3_s[:, kd, :],
                                  in_=Wg3[kd * 128:(kd + 1) * 128, :])
            wg4_s = const.tile([128, E], MMDT, name="wg4_s")
            nc.sync.dma_start(out=wg4_s, in_=Wg4)
            bg2_s = const.tile([128, 2], F32, name="bg2_s")
            for m in range(2):
                nc.sync.dma_start(out=bg2_s[:, m:m + 1],
                                  in_=bg2[m * 128:(m + 1) * 128, :])
            bg3_s = const.tile([128, 1], F32, name="bg3_s")
            nc.sync.dma_start(out=bg3_s, in_=bg3)
            bg4c_s = const.tile([E, 1], F32, name="bg4c_s")
            nc.sync.dma_start(out=bg4c_s, in_=bg4c)
            be1t_s = const.tile([128, HT, E], F32, name="be1t_s")
            for ht in range(HT):
                nc.sync.dma_start(out=be1t_s[:, ht, :],
                                  in_=be1T[ht * 128:(ht + 1) * 128, :])
            be2_s = const.tile([E, O], MMDT, name="be2_s")
            nc.sync.dma_start(out=be2_s, in_=be2)

            def lrelu_from_psum(dst, psum, bias_col):
                # dst = leaky_relu(psum + bias) with slope 0.01
                nc.scalar.activation(out=dst, in_=psum, func=AF.Identity,
                                     bias=bias_col, scale=1.0)
                nc.vector.scalar_tensor_tensor(out=dst, in0=dst, scalar=0.01,
                                               in1=dst, op0=ALU.mult,
                                               op1=ALU.max)

            def gating(bt, xs, gt_name="GT"):
                # 4-layer MLP; all activations [features_on_partitions, BT]
                pg = pbig.tile([128, BT], F32, name="pb")
                for kd in range(KD):
                    nc.tensor.matmul(pg, wg1_s[:, kd, :], xs[:, kd, :],
                                     start=(kd == 0), stop=(kd == KD - 1))
                g1 = gpool.tile([128, BT], MMDT, name="g1")
                lrelu_from_psum(g1, pg, bg1_s)

                g2 = gpool.tile([128, 2, BT], MMDT, name="g2")
                for m in range(2):
                    pg2 = pbig.tile([128, BT], F32, name="pb")
                    nc.tensor.matmul(pg2, wg2_s[:, m * 128:(m + 1) * 128],
                                     g1, start=True, stop=True)
                    lrelu_from_psum(g2[:, m, :], pg2, bg2_s[:, m:m + 1])

                pg3 = pbig.tile([128, BT], F32, name="pb")
                for kd in range(2):
                    nc.tensor.matmul(pg3, wg3_s[:, kd, :], g2[:, kd, :],
                                     start=(kd == 0), stop=(kd == 1))
                g3 = gpool.tile([128, BT], MMDT, name="g3")
                lrelu_from_psum(g3, pg3, bg3_s)

                # logits directly in [E, BT] layout via one matmul, softmax
                # across partitions (no max-subtract: |logits| < 20 always)
                pg4 = pbig.tile([128, BT], F32, name="pb")
                nc.tensor.matmul(pg4[:E, :], wg4_s, g3, start=True, stop=True)
                eg = smx.tile([E, BT], F32, name="eg")
                nc.scalar.activation(out=eg, in_=pg4[:E, :], func=AF.Exp,
                                     bias=(bg4c_s if has_bg4 else 0.0),
                                     scale=1.0)
                ssum = smx.tile([E, BT], F32, name="ss")
                nc.gpsimd.partition_all_reduce(
                    ssum, eg, channels=E, reduce_op=bass_isa.ReduceOp.add)
                rs = smx.tile([E, BT], F32, name="rs")
                nc.vector.reciprocal(rs, ssum)
                GTb = gtpool.tile([E, BT], BF16, name=gt_name + "b")
                nc.vector.tensor_mul(GTb, eg, rs)
                (nc.scalar if bt == 0 else nc.sync).dma_start(
                    out=gscr[bt], in_=GTb)
                GT = None
                if has_be2:
                    GT = gtpool.tile([E, BT], MMDT, name=gt_name)
                    nc.vector.tensor_mul(GT, eg, rs)
                return GTb, GT

            # first tile's gating runs before the big weight DMAs so the
            # PE starts immediately (its DMAs enqueued first above)
            gt0 = gating(0, xs0, gt_name="GT")

            # ---- expert weights: front-load we1 (needed first, e=0 almost
            #      immediately after gating), then interleave we2 so each
            #      we2_e lands well before its l2_block. Split across the
            #      SP HWDGE rings and SWDGE (gpsimd) rings; the
            #      latency-critical gscr/gb round trips ride the Activation
            #      HWDGE rings so they never queue behind the weight bulk.
            wdma = 0

            def wload(tile_, src, parts):
                nonlocal wdma
                for p in range(parts):
                    eng = nc.sync if wdma % 2 == 0 else nc.gpsimd
                    eng.dma_start(out=tile_[:, p, :], in_=src[p])
                    wdma += 1

            we1_s = [const.tile([128, KD, H], MMDT, name=f"we1_{e}")
                     for e in range(E)]
            we2_s = [const.tile([128, HT, O], MMDT, name=f"we2_{e}")
                     for e in range(E)]

            def w1src(e):
                return [We1[e, kd * 128:(kd + 1) * 128, :] for kd in range(KD)]

            def w2src(e):
                return [We2[e, ht * 128:(ht + 1) * 128, :] for ht in range(HT)]

            for e in range(4):
                wload(we1_s[e], w1src(e), KD)
            for e in range(4, E):
                wload(we2_s[e - 4], w2src(e - 4), HT)
                wload(we1_s[e], w1src(e), KD)
            for e in range(E - 4, E):
                wload(we2_s[e], w2src(e), HT)

            def l1_block(bt, e, xs, GTb):
                # gate row e broadcast over 128 partitions (bf16 DRAM round
                # trip on the Activation HWDGE rings, never behind weights)
                gb = gbpool.tile([128, BT], BF16, name="gb")
                eng = nc.scalar if bt == 0 else nc.sync
                eng.dma_start(
                    out=gb, in_=gscr[bt, e:e + 1, :].partition_broadcast(128))
                a1s = apool.tile([128, HT, BT], MMDT, name="a1s")
                for ht in range(HT):
                    p1 = pbig.tile([128, BT], F32, name="pb")
                    for kd in range(KD):
                        nc.tensor.matmul(
                            p1,
                            we1_s[e][:, kd, ht * 128:(ht + 1) * 128],
                            xs[:, kd, :],
                            start=(kd == 0), stop=(kd == KD - 1))
                    r1 = rpool.tile([128, BT], F32, name="r1")
                    nc.scalar.activation(out=r1, in_=p1, func=AF.Relu,
                                         bias=(be1t_s[:, ht, e:e + 1]
                                               if has_be1 else 0.0),
                                         scale=1.0)
                    eng = nc.vector if ht < 2 else nc.gpsimd
                    eng.tensor_mul(a1s[:, ht, :], r1, gb)
                return a1s

            def l2_block(e, a1s, po, first, last):
                for ht in range(HT):
                    for ot in range(OT):
                        nc.tensor.matmul(
                            po[:, ot, :],
                            we2_s[e][:, ht, ot * 128:(ot + 1) * 128],
                            a1s[:, ht, :],
                            start=(first and ht == 0),
                            stop=(last and not has_be2 and ht == HT - 1))

            def gating_stages(bt, xs, holder):
                # the gating() pipeline split into 5 emission stages so the
                # scheduler can interleave expert matmuls with each serial
                # MM -> ACT -> DVE link of the MLP chain
                def s1():
                    pg = pbig.tile([128, BT], F32, name="pb")
                    for kd in range(KD):
                        nc.tensor.matmul(pg, wg1_s[:, kd, :], xs[:, kd, :],
                                         start=(kd == 0), stop=(kd == KD - 1))
                    g1 = gpool.tile([128, BT], MMDT, name="g1")
                    lrelu_from_psum(g1, pg, bg1_s)
                    holder["g1"] = g1

                def s2():
                    g1 = holder["g1"]
                    g2 = gpool.tile([128, 2, BT], MMDT, name="g2")
                    for m in range(2):
                        pg2 = pbig.tile([128, BT], F32, name="pb")
                        nc.tensor.matmul(pg2,
                                         wg2_s[:, m * 128:(m + 1) * 128],
                                         g1, start=True, stop=True)
                        lrelu_from_psum(g2[:, m, :], pg2, bg2_s[:, m:m + 1])
                    holder["g2"] = g2

                def s3():
                    g2 = holder["g2"]
                    pg3 = pbig.tile([128, BT], F32, name="pb")
                    for kd in range(2):
                        nc.tensor.matmul(pg3, wg3_s[:, kd, :], g2[:, kd, :],
                                         start=(kd == 0), stop=(kd == 1))
                    g3 = gpool.tile([128, BT], MMDT, name="g3")
                    lrelu_from_psum(g3, pg3, bg3_s)
                    holder["g3"] = g3

                def s4():
                    # logits directly in [E, BT] via one matmul + exp
                    g3 = holder["g3"]
                    pg4 = pbig.tile([128, BT], F32, name="pb")
                    nc.tensor.matmul(pg4[:E, :], wg4_s, g3,
                                     start=True, stop=True)
                    eg = smx.tile([E, BT], F32, name="eg")
                    nc.scalar.activation(out=eg, in_=pg4[:E, :], func=AF.Exp,
                                         bias=(bg4c_s if has_bg4 else 0.0),
                                         scale=1.0)
                    holder["eg"] = eg

                def s5():
                    ssum = smx.tile([E, BT], F32, name="ss")
                    nc.gpsimd.partition_all_reduce(
                        ssum, holder["eg"], channels=E,
                        reduce_op=bass_isa.ReduceOp.add)
                    holder["ss"] = ssum

                def s6():
                    rs = smx.tile([E, BT], F32, name="rs")
                    nc.vector.reciprocal(rs, holder["ss"])
                    GTb = gtpool.tile([E, BT], BF16, name="GTb")
                    nc.vector.tensor_mul(GTb, holder["eg"], rs)
                    nc.sync.dma_start(out=gscr[bt], in_=GTb)
                    GT = None
                    if has_be2:
                        GT = gtpool.tile([E, BT], MMDT, name="GT")
                        nc.vector.tensor_mul(GT, holder["eg"], rs)
                    holder["GT"] = (GTb, GT)

                return [s1, s2, s3, s4, s5, s6]

            def run_body():
                xs_next, gt_next = xs0, gt0
                for bt in range(NBT):
                    xs, (GTb, GT) = xs_next, gt_next
                    po = pout.tile([128, OT, BT], F32, name="po")
                    a1_prev = None
                    stages, holder = None, None
                    for e in range(E):
                        a1_cur = l1_block(bt, e, xs, GTb)
                        if e > 0:
                            l2_block(e - 1, a1_prev, po, first=(e == 1),
                                     last=False)
                        if bt + 1 < NBT:
                            if e == 2:
                                xs_next = load_x(bt + 1)
                                holder = {}
                                stages = gating_stages(bt + 1, xs_next,
                                                       holder)
                            if stages and 2 <= e < 2 + len(stages):
                                stages[e - 2]()
                        a1_prev = a1_cur
                    if holder is not None:
                        gt_next = holder["GT"]
                    l2_block(E - 1, a1_prev, po, first=False, last=True)
                    for ot in range(OT):
                        if has_be2:
                            nc.tensor.matmul(po[:, ot, :],
                                             be2_s[:, ot * 128:(ot + 1) * 128],
                                             GT, start=False, stop=True)
                        osb = opool.tile([128, BT], F32, name="osb")
                        for hb in range(2):
                            cs = slice(hb * (BT // 2), (hb + 1) * (BT // 2))
                            nc.vector.tensor_copy(out=osb[:, cs],
                                                  in_=po[:, ot, cs])
                            nc.sync.dma_start(
                                out=outT[ot * 128:(ot + 1) * 128,
                                         bt * BT + hb * (BT // 2):
                                         bt * BT + (hb + 1) * (BT // 2)],
                                in_=osb[:, cs])

            if n_reps > 1:
                if unroll:
                    for _ in range(n_reps):
                        run_body()
                else:
                    with tc.For_i(0, n_reps, 1):
                        run_body()
            else:
                run_body()

    nc.compile()
    return nc


_program_cache = {}


def get_program(has_be1=False, has_be2=False, has_bg4=False):
    key = (has_be1, has_be2, has_bg4)
    if key not in _program_cache:
        _program_cache[key] = build_program(
            has_be1=has_be1, has_be2=has_be2, has_bg4=has_bg4)
    return _program_cache[key]


def make_in_maps(inputs):
    f = lambda a: np.ascontiguousarray(np.asarray(a, dtype=np.float32))
    x = f(inputs["x"])
    shared = {
        "We1": f(inputs["We1"]),
        "We2": f(inputs["We2"]),
        "be1T": f(inputs["be1"]).T.copy(),
        "be2": f(inputs["be2"]),
        "Wg1": f(inputs["Wg1"]),
        "Wg2": f(inputs["Wg2"]),
        "Wg3": f(inputs["Wg3"]),
        "Wg4": f(inputs["Wg4"]),
        "bg1": f(inputs["bg1"]).reshape(128, 1),
        "bg2": f(inputs["bg2"]).reshape(256, 1),
        "bg3": f(inputs["bg3"]).reshape(128, 1),
        "bg4c": f(inputs["bg4"]).reshape(E, 1),
    }
    in_maps = []
    for c in range(NCORES):
        m = dict(shared)
        m["xT"] = np.ascontiguousarray(x[c * BC:(c + 1) * BC, :].T)
        in_maps.append(m)
    return in_maps


def kernel(**inputs) -> np.ndarray:
    nc = get_program(has_be1=bool(np.any(np.asarray(inputs["be1"]))),
                     has_be2=bool(np.any(np.asarray(inputs["be2"]))),
                     has_bg4=bool(np.any(np.asarray(inputs["bg4"]))))
    in_maps = make_in_maps(inputs)
    res = run_bass_kernel_spmd(nc, in_maps, core_ids=list(range(NCORES)))
    out = np.empty((B, O), dtype=np.float32)
    for c in range(NCORES):
        out[c * BC:(c + 1) * BC, :] = res.results[c]["outT"].T
    return out



# revision 20
# speedup vs baseline: 1.3763x; 1.3763x over previous
"""Trainium2 Bass kernel for the dense-MoE problem (nn_MoE_20899310862533).

Contract: kernel(**inputs) takes the FULL unsharded inputs (keys as in
reference.setup_inputs()) and returns the FULL [32768, 256] float32 output.

Strategy: data-parallel over the batch across 8 NeuronCores (weights
replicated, no collectives). On chip everything is kept feature-major
([feature, batch]) so every matmul has its contraction dim on partitions and
a 512-wide moving dim; the host passes x pre-transposed per shard and
transposes the per-core output back.

Per core (batch shard 4096, processed in 8 tiles of 512):
  gating MLP (4 layers, LeakyReLU 0.01) -> softmax -> gates G^T [16, 512]
  experts, all fused in SBUF/PSUM:
      act1_e   = relu(x @ We1[e] + be1[e])          [512h, 512b] per expert
      act1s_e  = act1_e * g_e (broadcast over h)
      psum_out += We2[e].T-contraction with act1s_e  (accumulated over e)
      psum_out += be2.T @ G^T                        (gates-weighted bias)
  Gate rows are broadcast across 128 partitions with a DRAM round-trip
  broadcast DMA. Matmuls run as float32r (full-rate fp32 PE mode); the PE
  is the bottleneck engine at ~98% occupancy of the fp32r roofline
  (fp8/bf16/top-k approximations all fail the 2e-2 accuracy gate; fp8
  DoubleRow is only 2x and needs a 3x-matmul residual scheme to hit the
  tolerance, which is net slower).

  Prologue is latency-tuned for single-shot execution: gating weights and
  the first x tile are DMA'd first (PE starts ~4us in), expert weights are
  front-loaded we1-first then interleaved with we2 in consumption order
  across the SP-HWDGE and SWDGE rings, and the first batch-tile's gate
  broadcasts ride the Activation-HWDGE rings so they never queue behind
  the 16.8MB weight bulk. Output DMAs are split in column halves so the
  final drain overlaps the copy.
"""

import numpy as np

import concourse.bass as bass
import concourse.mybir as mybir
import concourse.tile as tile
from concourse import bacc
from concourse.bass_utils import run_bass_kernel_spmd

F32 = mybir.dt.float32
F32R = mybir.dt.float32r
BF16 = mybir.dt.bfloat16
AF = mybir.ActivationFunctionType
ALU = mybir.AluOpType
AX = mybir.AxisListType

B, D, H, O, E = 32768, 256, 512, 256, 16
NCORES = 8
BC = B // NCORES      # 4096 batch rows per core
BT = 512              # batch tile (moving/free dim)
NBT = BC // BT        # 8
KD = D // 128         # 2 contraction tiles for D
HT = H // 128         # 4 tiles for H
OT = O // 128         # 2 tiles for O

MMDT = F32R           # matmul operand dtype view (F32R = fast, F32 = exact)


def build_program(n_reps=1, has_be1=False, has_be2=False, has_bg4=False,
                  psum_cfg=(4, 2), unroll=False):
    nc = bacc.Bacc("TRN2", target_bir_lowering=False, debug=False,
                   num_devices=NCORES)

    def din(name, shape, dt=F32):
        return nc.dram_tensor(name, shape, dt, kind="ExternalInput").ap()

    xT = din("xT", [D, BC], MMDT)
    We1 = din("We1", [E, D, H], MMDT)
    We2 = din("We2", [E, H, O], MMDT)
    be1T = din("be1T", [H, E])
    be2 = din("be2", [E, O], MMDT)
    Wg1 = din("Wg1", [D, 128], MMDT)
    Wg2 = din("Wg2", [128, 256], MMDT)
    Wg3 = din("Wg3", [256, 128], MMDT)
    Wg4 = din("Wg4", [128, E], MMDT)
    bg1 = din("bg1", [128, 1])
    bg2 = din("bg2", [256, 1])
    bg3 = din("bg3", [128, 1])
    bg4r = din("bg4r", [1, E], MMDT)
    ones1 = din("ones1", [1, 128], MMDT)
    ident = din("ident", [128, 128])
    outT = nc.dram_tensor("outT", [O, BC], F32, kind="ExternalOutput").ap()
    gscr = nc.dram_tensor("gscr", [NBT, E, BT], MMDT, kind="Internal").ap()

    with tile.TileContext(nc) as tc:
        from contextlib import ExitStack
        with ExitStack() as ctx:
            const = ctx.enter_context(tc.tile_pool(name="const", bufs=1))
            xpool = ctx.enter_context(tc.tile_pool(name="x", bufs=2))
            gpool = ctx.enter_context(tc.tile_pool(name="gate", bufs=1))
            smx = ctx.enter_context(tc.tile_pool(name="smx", bufs=1))
            gtpool = ctx.enter_context(tc.tile_pool(name="gt", bufs=2))
            gbpool = ctx.enter_context(tc.tile_pool(name="gb", bufs=3))
            rpool = ctx.enter_context(tc.tile_pool(name="r1", bufs=3))
            apool = ctx.enter_context(tc.tile_pool(name="a1s", bufs=2))
            opool = ctx.enter_context(tc.tile_pool(name="osb", bufs=2))
            pbig = ctx.enter_context(
                tc.tile_pool(name="pbig", bufs=psum_cfg[0], space="PSUM"))
            pout = ctx.enter_context(
                tc.tile_pool(name="pout", bufs=psum_cfg[1], space="PSUM"))

            def load_x(bt, name="xs", split=1):
                xs = xpool.tile([128, KD, BT], MMDT, name=name)
                w = BT // split
                for kd in range(KD):
                    for s in range(split):
                        nc.sync.dma_start(
                            out=xs[:, kd, s * w:(s + 1) * w],
                            in_=xT[kd * 128:(kd + 1) * 128,
                                   bt * BT + s * w:bt * BT + (s + 1) * w])
                return xs

            # ---- gating-critical constants + first x tile first, so the
            #      PE starts as early as possible ----
            wg1_s = const.tile([128, KD, 128], MMDT, name="wg1_s")
            for kd in range(KD):
                nc.sync.dma_start(out=wg1_s[:, kd, :],
                                  in_=Wg1[kd * 128:(kd + 1) * 128, :])
            bg1_s = const.tile([128, 1], F32, name="bg1_s")
            nc.sync.dma_start(out=bg1_s, in_=bg1)
            xs0 = load_x(0, name="xs0", split=2)
            wg2_s = const.tile([128, 256], MMDT, name="wg2_s")
            nc.sync.dma_start(out=wg2_s, in_=Wg2)
            wg3_s = const.tile([128, KD, 128], MMDT, name="wg3_s")
            for kd in range(KD):
                nc.sync.dma_start(out=wg3_s[:, kd, :],
                                  in_=Wg3[kd * 128:(kd + 1) * 128, :])
            wg4_s = const.tile([128, E], MMDT, name="wg4_s")
            nc.sync.dma_start(out=wg4_s, in_=Wg4)
            bg2_s = const.tile([128, 2], F32, name="bg2_s")
            for m in range(2):
                nc.sync.dma_start(out=bg2_s[:, m:m + 1],
                                  in_=bg2[m * 128:(m + 1) * 128, :])
            bg3_s = const.tile([128, 1], F32, name="bg3_s")
            nc.sync.dma_start(out=bg3_s, in_=bg3)
            bg4r_s = const.tile([1, E], MMDT, name="bg4r_s")
            nc.sync.dma_start(out=bg4r_s, in_=bg4r)
            be1t_s = const.tile([128, HT, E], F32, name="be1t_s")
            for ht in range(HT):
                nc.sync.dma_start(out=be1t_s[:, ht, :],
                                  in_=be1T[ht * 128:(ht + 1) * 128, :])
            be2_s = const.tile([E, O], MMDT, name="be2_s")
            nc.sync.dma_start(out=be2_s, in_=be2)
            id_s = const.tile([128, 128], F32, name="id_s")
            nc.sync.dma_start(out=id_s, in_=ident)
            ones_s = const.tile([1, 128], MMDT, name="ones_s")
            nc.sync.dma_start(out=ones_s, in_=ones1)

            def lrelu_from_psum(dst, psum, bias_col):
                # dst = leaky_relu(psum + bias) with slope 0.01
                nc.scalar.activation(out=dst, in_=psum, func=AF.Identity,
                                     bias=bias_col, scale=1.0)
                nc.vector.scalar_tensor_tensor(out=dst, in0=dst, scalar=0.01,
                                               in1=dst, op0=ALU.mult,
                                               op1=ALU.max)

            def gating(bt, xs, gt_name="GT"):
                # 4-layer MLP; all activations [features_on_partitions, BT]
                pg = pbig.tile([128, BT], F32, name="pb")
                for kd in range(KD):
                    nc.tensor.matmul(pg, wg1_s[:, kd, :], xs[:, kd, :],
                                     start=(kd == 0), stop=(kd == KD - 1))
                g1 = gpool.tile([128, BT], MMDT, name="g1")
                lrelu_from_psum(g1, pg, bg1_s)

                g2 = gpool.tile([128, 2, BT], MMDT, name="g2")
                for m in range(2):
                    pg2 = pbig.tile([128, BT], F32, name="pb")
                    nc.tensor.matmul(pg2, wg2_s[:, m * 128:(m + 1) * 128],
                                     g1, start=True, stop=True)
                    lrelu_from_psum(g2[:, m, :], pg2, bg2_s[:, m:m + 1])

                pg3 = pbig.tile([128, BT], F32, name="pb")
                for kd in range(2):
                    nc.tensor.matmul(pg3, wg3_s[:, kd, :], g2[:, kd, :],
                                     start=(kd == 0), stop=(kd == 1))
                g3 = gpool.tile([128, BT], MMDT, name="g3")
                lrelu_from_psum(g3, pg3, bg3_s)

                # logits in [batch, E] layout (softmax along free dim), then
                # transpose each 128-batch block onto partitions 0..15
                GT = gtpool.tile([E, BT], MMDT, name=gt_name)
                for sb in range(BT // 128):
                    pg4 = pbig.tile([128, BT], F32, name="pb")
                    nc.tensor.matmul(pg4[:, :E],
                                     g3[:, sb * 128:(sb + 1) * 128],
                                     wg4_s, start=True, stop=not has_bg4)
                    if has_bg4:
                        nc.tensor.matmul(pg4[:, :E], ones_s, bg4r_s,
                                         start=False, stop=True)
                    nmx = smx.tile([128, 1], F32, name="nmx")
                    nc.vector.reduce_max(nmx, pg4[:, :E], AX.X, negate=True)
                    eg = smx.tile([128, E], F32, name="eg")
                    nc.scalar.activation(out=eg, in_=pg4[:, :E], func=AF.Exp,
                                         bias=nmx, scale=1.0)
                    sm = smx.tile([128, 1], F32, name="sm")
                    nc.vector.reduce_sum(sm, eg, AX.X)
                    rs = smx.tile([128, 1], F32, name="rs")
                    nc.vector.reciprocal(rs, sm)
                    gg = smx.tile([128, E], F32, name="gg")
                    nc.vector.tensor_scalar_mul(gg, eg, rs)
                    ptr = pbig.tile([128, BT], F32, name="pb")
                    nc.tensor.transpose(ptr[:E, :128], gg, id_s)
                    nc.vector.tensor_copy(out=GT[:, sb * 128:(sb + 1) * 128],
                                          in_=ptr[:E, :128])
                (nc.scalar if bt == 0 else nc.sync).dma_start(
                    out=gscr[bt], in_=GT)
                return GT

            # first tile's gating runs before the big weight DMAs so the
            # PE starts immediately (its DMAs enqueued first above)
            gt0 = gating(0, xs0, gt_name="GT")

            # ---- expert weights: front-load we1 (needed first, e=0 almost
            #      immediately after gating), then interleave we2 so each
            #      we2_e lands well before its l2_block. Split across the
            #      SP HWDGE rings and SWDGE (gpsimd) rings; the
            #      latency-critical gscr/gb round trips ride the Activation
            #      HWDGE rings so they never queue behind the weight bulk.
            wdma = 0

            def wload(tile_, src, parts):
                nonlocal wdma
                for p in range(parts):
                    eng = nc.sync if wdma % 2 == 0 else nc.gpsimd
                    eng.dma_start(out=tile_[:, p, :], in_=src[p])
                    wdma += 1

            we1_s = [const.tile([128, KD, H], MMDT, name=f"we1_{e}")
                     for e in range(E)]
            we2_s = [const.tile([128, HT, O], MMDT, name=f"we2_{e}")
                     for e in range(E)]

            def w1src(e):
                return [We1[e, kd * 128:(kd + 1) * 128, :] for kd in range(KD)]

            def w2src(e):
                return [We2[e, ht * 128:(ht + 1) * 128, :] for ht in range(HT)]

            for e in range(4):
                wload(we1_s[e], w1src(e), KD)
            for e in range(4, E):
                wload(we2_s[e - 4], w2src(e - 4), HT)
                wload(we1_s[e], w1src(e), KD)
            for e in range(E - 4, E):
                wload(we2_s[e], w2src(e), HT)

            def l1_block(bt, e, xs, GT):
                # gate row e broadcast over 128 partitions (DRAM round trip;
                # bt0 rides the Activation HWDGE rings, never behind weights)
                gb = gbpool.tile([128, BT], MMDT, name="gb")
                eng = nc.scalar if bt == 0 else nc.sync
                eng.dma_start(
                    out=gb, in_=gscr[bt, e:e + 1, :].partition_broadcast(128))
                a1s = apool.tile([128, HT, BT], MMDT, name="a1s")
                for ht in range(HT):
                    p1 = pbig.tile([128, BT], F32, name="pb")
                    for kd in range(KD):
                        nc.tensor.matmul(
                            p1,
                            we1_s[e][:, kd, ht * 128:(ht + 1) * 128],
                            xs[:, kd, :],
                            start=(kd == 0), stop=(kd == KD - 1))
                    r1 = rpool.tile([128, BT], F32, name="r1")
                    nc.scalar.activation(out=r1, in_=p1, func=AF.Relu,
                                         bias=(be1t_s[:, ht, e:e + 1]
                                               if has_be1 else 0.0),
                                         scale=1.0)
                    eng = nc.vector if ht < 2 else nc.gpsimd
                    eng.tensor_mul(a1s[:, ht, :], r1, gb)
                return a1s

            def l2_block(e, a1s, po, first, last):
                for ht in range(HT):
                    for ot in range(OT):
                        nc.tensor.matmul(
                            po[:, ot, :],
                            we2_s[e][:, ht, ot * 128:(ot + 1) * 128],
                            a1s[:, ht, :],
                            start=(first and ht == 0),
                            stop=(last and not has_be2 and ht == HT - 1))

            def gating_stages(bt, xs, holder):
                # the gating() pipeline split into 5 emission stages so the
                # scheduler can interleave expert matmuls with each serial
                # MM -> ACT -> DVE link of the MLP chain
                def s1():
                    pg = pbig.tile([128, BT], F32, name="pb")
                    for kd in range(KD):
                        nc.tensor.matmul(pg, wg1_s[:, kd, :], xs[:, kd, :],
                                         start=(kd == 0), stop=(kd == KD - 1))
                    g1 = gpool.tile([128, BT], MMDT, name="g1")
                    lrelu_from_psum(g1, pg, bg1_s)
                    holder["g1"] = g1

                def s2():
                    g1 = holder["g1"]
                    g2 = gpool.tile([128, 2, BT], MMDT, name="g2")
                    for m in range(2):
                        pg2 = pbig.tile([128, BT], F32, name="pb")
                        nc.tensor.matmul(pg2,
                                         wg2_s[:, m * 128:(m + 1) * 128],
                                         g1, start=True, stop=True)
                        lrelu_from_psum(g2[:, m, :], pg2, bg2_s[:, m:m + 1])
                    holder["g2"] = g2

                def s3():
                    g2 = holder["g2"]
                    pg3 = pbig.tile([128, BT], F32, name="pb")
                    for kd in range(2):
                        nc.tensor.matmul(pg3, wg3_s[:, kd, :], g2[:, kd, :],
                                         start=(kd == 0), stop=(kd == 1))
                    g3 = gpool.tile([128, BT], MMDT, name="g3")
                    lrelu_from_psum(g3, pg3, bg3_s)
                    holder["g3"] = g3
                    holder["GT"] = gtpool.tile([E, BT], MMDT, name="GT")

                def softmax_block(sb):
                    # logits matmul + softmax; the PE transpose is deferred
                    # to s6 so it never blocks the in-order PE stream
                    g3 = holder["g3"]
                    pg4 = pbig.tile([128, BT], F32, name="pb")
                    nc.tensor.matmul(pg4[:, :E],
                                     g3[:, sb * 128:(sb + 1) * 128],
                                     wg4_s, start=True, stop=not has_bg4)
                    if has_bg4:
                        nc.tensor.matmul(pg4[:, :E], ones_s, bg4r_s,
                                         start=False, stop=True)
                    nmx = smx.tile([128, 1], F32, name="nmx")
                    nc.vector.reduce_max(nmx, pg4[:, :E], AX.X, negate=True)
                    eg = smx.tile([128, E], F32, name="eg")
                    nc.scalar.activation(out=eg, in_=pg4[:, :E], func=AF.Exp,
                                         bias=nmx, scale=1.0)
                    sm = smx.tile([128, 1], F32, name="sm")
                    nc.vector.reduce_sum(sm, eg, AX.X)
                    rs = smx.tile([128, 1], F32, name="rs")
                    nc.vector.reciprocal(rs, sm)
                    gg = smx.tile([128, E], F32, name="gg")
                    nc.vector.tensor_scalar_mul(gg, eg, rs)
                    holder[f"gg{sb}"] = gg

                def s4():
                    softmax_block(0)
                    softmax_block(1)

                def s5():
                    softmax_block(2)
                    softmax_block(3)

                def s6():
                    GT = holder["GT"]
                    for sb in range(4):
                        ptr = pbig.tile([128, BT], F32, name="pb")
                        nc.tensor.transpose(ptr[:E, :128],
                                            holder[f"gg{sb}"], id_s)
                        nc.vector.tensor_copy(
                            out=GT[:, sb * 128:(sb + 1) * 128],
                            in_=ptr[:E, :128])
                    nc.sync.dma_start(out=gscr[bt], in_=GT)

                return [s1, s2, s3, s4, s5, s6]

            def run_body():
                xs_next, gt_next = xs0, gt0
                for bt in range(NBT):
                    xs, GT = xs_next, gt_next
                    po = pout.tile([128, OT, BT], F32, name="po")
                    a1_prev = None
                    stages, holder = None, None
                    for e in range(E):
                        a1_cur = l1_block(bt, e, xs, GT)
                        if e > 0:
                            l2_block(e - 1, a1_prev, po, first=(e == 1),
                                     last=False)
                        if bt + 1 < NBT:
                            if e == 2:
                                xs_next = load_x(bt + 1)
                                holder = {}
                                stages = gating_stages(bt + 1, xs_next,
                                                       holder)
                            if stages and 2 <= e < 2 + len(stages):
                                stages[e - 2]()
                        a1_prev = a1_cur
                    if holder is not None:
                        gt_next = holder["GT"]
                    l2_block(E - 1, a1_prev, po, first=False, last=True)
                    for ot in range(OT):
                        if has_be2:
                            nc.tensor.matmul(po[:, ot, :],
                                             be2_s[:, ot * 128:(ot + 1) * 128],
                                             GT, start=False, stop=True)
                        osb = opool.tile([128, BT], F32, name="osb")
                        for hb in range(2):
                            cs = slice(hb * (BT // 2), (hb + 1) * (BT // 2))
                            nc.vector.tensor_copy(out=osb[:, cs],
                                                  in_=po[:, ot, cs])
                            nc.sync.dma_start(
                                out=outT[ot * 128:(ot + 1) * 128,
                                         bt * BT + hb * (BT // 2):
                                         bt * BT + (hb + 1) * (BT // 2)],
                                in_=osb[:, cs])

            if n_reps > 1:
                if unroll:
                    for _ in range(n_reps):
                        run_body()
                else:
                    with tc.For_i(0, n_reps, 1):
                        run_body()
            else:
                run_body()

    nc.compile()
    return nc


_program_cache = {}


def get_program(has_be1=False, has_be2=False, has_bg4=False):
    key = (has_be1, has_be2, has_bg4)
    if key not in _program_cache:
        _program_cache[key] = build_program(
            has_be1=has_be1, has_be2=has_be2, has_bg4=has_bg4)
    return _program_cache[key]


def make_in_maps(inputs):
    f = lambda a: np.ascontiguousarray(np.asarray(a, dtype=np.float32))
    x = f(inputs["x"])
    shared = {
        "We1": f(inputs["We1"]),
        "We2": f(inputs["We2"]),
        "be1T": f(inputs["be1"]).T.copy(),
        "be2": f(inputs["be2"]),
        "Wg1": f(inputs["Wg1"]),
        "Wg2": f(inputs["Wg2"]),
        "Wg3": f(inputs["Wg3"]),
        "Wg4": f(inputs["Wg4"]),
        "bg1": f(inputs["bg1"]).reshape(128, 1),
        "bg2": f(inputs["bg2"]).reshape(256, 1),
        "bg3": f(inputs["bg3"]).reshape(128, 1),
        "bg4r": f(inputs["bg4"]).reshape(1, E),
        "ident": np.eye(128, dtype=np.float32),
        "ones1": np.ones((1, 128), dtype=np.float32),
    }
    in_maps = []
    for c in range(NCORES):
        m = dict(shared)
        m["xT"] = np.ascontiguousarray(x[c * BC:(c + 1) * BC, :].T)
        in_maps.append(m)
    return in_maps


def kernel(**inputs) -> np.ndarray:
    nc = get_program(has_be1=bool(np.any(np.asarray(inputs["be1"]))),
                     has_be2=bool(np.any(np.asarray(inputs["be2"]))),
                     has_bg4=bool(np.any(np.asarray(inputs["bg4"]))))
    in_maps = make_in_maps(inputs)
    res = run_bass_kernel_spmd(nc, in_maps, core_ids=list(range(NCORES)))
    out = np.empty((B, O), dtype=np.float32)
    for c in range(NCORES):
        out[c * BC:(c + 1) * BC, :] = res.results[c]["outT"].T
    return out

